# revision 1
# baseline (speedup 1.0000x reference)
"""2-layer GCN (GCNConv x2) on 8 trn2 NeuronCores.

Strategy (dst-node sharding, ELL gather-aggregation):
  out = D^-1/2 (A+I) D^-1/2 (X W) + b  per layer; by associativity we
  aggregate width-64 tables in BOTH layers:
    L1: agg1 = Ahat @ (dinv*x)         ; q = dinv_d*agg1 @ W1 + b1; h = relu(q)
    L2: hw = h @ W2; agg2 = Ahat_noself-ish via table2 = dinv*hw
        out = dinv_d*(sum over nbrs incl self of table2) + b2
  Each core owns 6250 dst nodes (padded to 6272 = 49 tiles x 128).
  Host preprocessing (graph partitioner): degrees, dinv, per-core
  degree-sorted ELL neighbor lists; padding slots point at zero rows.
  Gathers: gpsimd.indirect_dma_start with [128,1] per-partition offsets
  (one 128-row x 256B tile per call).  Inter-layer exchange: AllGather.
"""
import numpy as np

N_NODES = 50000
DIN, DH, DOUT = 64, 128, 64
NCORES = 8
NLOC = N_NODES // NCORES            # 6250
T_LOC = 49                          # tiles of 128 dst slots
SLOTS = T_LOC * 128                 # 6272
NT_X = 391                          # node tiles for x' prep (391*128 = 50048)
XPAD = NT_X * 128                   # padded x rows / table1 rows
XCH = 64                            # x' prep chunk (tiles per chunk)
TAB2 = NCORES * SLOTS               # 50176 table2 rows
P = 128

_cache = {}


def _host_prep(edge_index):
    src = edge_index[0].astype(np.int64)
    dst = edge_index[1].astype(np.int64)
    deg = np.bincount(dst, minlength=N_NODES).astype(np.int64) + 1
    dinv = np.zeros(N_NODES, np.float32)
    dinv[:] = 1.0 / np.sqrt(deg.astype(np.float64))

    order = np.argsort(dst, kind="stable")
    ssrc = src[order]
    starts = np.searchsorted(dst[order], np.arange(N_NODES))
    ends = np.searchsorted(dst[order], np.arange(N_NODES) + 1)
    cnt = (ends - starts).astype(np.int64)          # in-degree (no self)

    perms = []          # per core: slot -> node id (len SLOTS, -1 dummy)
    Ks = np.zeros((NCORES, T_LOC), np.int64)
    for c in range(NCORES):
        lo = c * NLOC
        nodes = np.arange(lo, lo + NLOC)
        o = np.argsort(-cnt[nodes], kind="stable")
        perm = np.full(SLOTS, -1, np.int64)
        perm[:NLOC] = nodes[o]
        perms.append(perm)
        ell = cnt[perm[:NLOC]] + 1                   # + self loop
        for t in range(T_LOC):
            seg = ell[t * 128:(t + 1) * 128]
            Ks[c, t] = seg.max() if len(seg) else 1
    Kt = Ks.max(axis=0)                              # SPMD: common per-tile K
    off = np.concatenate([[0], np.cumsum(Kt)])
    SK = int(off[-1])

    # slot position of each node (for table2 row mapping)
    slotpos = np.zeros(N_NODES, np.int64)
    for c in range(NCORES):
        p = perms[c]
        slotpos[p[:NLOC]] = np.arange(NLOC)
    row2_of_node = (np.arange(N_NODES) // NLOC) * SLOTS + slotpos

    IDX1 = np.full((NCORES, P, SK), XPAD - 128, np.int32)   # pad -> zero row
    IDX2 = np.full((NCORES, P, SK), SLOTS - 1, np.int32)    # pad -> dummy slot0 row
    # zero rows: table1 rows >= 50000 are zero (dinv pad = 0). use row 50000.
    ZROW1 = N_NODES
    ZROW2 = NLOC          # slot NLOC of core 0 is dummy (zeroed via dinv=0)
    IDX1[:] = ZROW1
    IDX2[:] = ZROW2
    dinv_loc = np.zeros((NCORES, P, T_LOC), np.float32)
    for c in range(NCORES):
        perm = perms[c]
        for t in range(T_LOC):
            base = t * 128
            K = int(Kt[t])
            for p in range(128):
                n = perm[base + p]
                if n < 0:
                    continue
                dinv_loc[c, p, t] = dinv[n]
                nb = ssrc[starts[n]:ends[n]]
                k = len(nb)
                IDX1[c, p, off[t]:off[t] + k] = nb
                IDX1[c, p, off[t] + k] = n            # self loop
                IDX2[c, p, off[t]:off[t] + k] = row2_of_node[nb]
                IDX2[c, p, off[t] + k] = row2_of_node[n]

    dinv_all = np.zeros((P, NT_X), np.float32)
    idx = np.arange(XPAD)
    valid = idx < N_NODES
    dinv_all[idx[valid] % 128, idx[valid] // 128] = dinv[idx[valid]]
    return dict(Kt=Kt.astype(int), off=off.astype(int), SK=SK, IDX1=IDX1,
                IDX2=IDX2, dinv_loc=dinv_loc, dinv_all=dinv_all, perms=perms)


def _build_nc(Kt, off, SK, rep=1):
    import concourse.bass as bass
    import concourse.bacc as bacc
    import concourse.mybir as mybir
    import concourse.tile as tile
    from concourse.masks import make_identity

    f32 = mybir.dt.float32
    nc = bacc.Bacc("TRN2", target_bir_lowering=False)
    x_in = nc.dram_tensor("x_in", [XPAD, DIN], f32, kind="ExternalInput")
    w1 = nc.dram_tensor("w1", [DIN, DH], f32, kind="ExternalInput")
    b1 = nc.dram_tensor("b1", [DH, 1], f32, kind="ExternalInput")
    w2 = nc.dram_tensor("w2", [DH, DOUT], f32, kind="ExternalInput")
    b2 = nc.dram_tensor("b2", [1, DOUT], f32, kind="ExternalInput")
    idx1 = nc.dram_tensor("idx1", [P, SK], mybir.dt.int32, kind="ExternalInput")
    idx2 = nc.dram_tensor("idx2", [P, SK], mybir.dt.int32, kind="ExternalInput")
    dinv_loc_d = nc.dram_tensor("dinv_loc", [P, T_LOC], f32, kind="ExternalInput")
    dinv_all_d = nc.dram_tensor("dinv_all", [P, NT_X], f32, kind="ExternalInput")
    out_d = nc.dram_tensor("out", [P, T_LOC * DOUT], f32, kind="ExternalOutput")

    table1 = nc.dram_tensor("table1", [XPAD, DIN], f32)
    slice2 = nc.dram_tensor("slice2", [SLOTS, DOUT], f32)
    table2 = nc.dram_tensor("table2", [TAB2, DOUT], f32)

    Kmax = int(max(Kt))
    with tile.TileContext(nc) as tc:
        with (
            tc.tile_pool(name="const", bufs=1) as cp,
            tc.tile_pool(name="xc", bufs=2) as xp,
            tc.tile_pool(name="g", bufs=2) as gp,
            tc.tile_pool(name="big", bufs=1) as bp,
            tc.tile_pool(name="ps", bufs=2, space="PSUM") as pp,
            tc.tile_pool(name="ps2", bufs=2, space="PSUM") as pp2,
        ):
            ident = cp.tile([P, P], f32)
            make_identity(nc, ident[:])
            w1_sb = cp.tile([DIN, DH], f32)
            w2_sb = cp.tile([DH, DOUT], f32)
            b1_sb = cp.tile([DH, 1], f32)
            b2_sb = cp.tile([P, DOUT], f32)
            dl_sb = cp.tile([P, T_LOC], f32)
            da_sb = cp.tile([P, NT_X], f32)
            i1_sb = cp.tile([P, SK], mybir.dt.int32)
            i2_sb = cp.tile([P, SK], mybir.dt.int32)
            nc.gpsimd.dma_start(w1_sb[:], w1[:])
            nc.gpsimd.dma_start(w2_sb[:], w2[:])
            nc.gpsimd.dma_start(b1_sb[:], b1[:])
            nc.gpsimd.dma_start(b2_sb[:], b2[:].to_broadcast([P, DOUT]))
            nc.gpsimd.dma_start(dl_sb[:], dinv_loc_d[:])
            nc.gpsimd.dma_start(da_sb[:], dinv_all_d[:])
            nc.gpsimd.dma_start(i1_sb[:], idx1[:])
            nc.gpsimd.dma_start(i2_sb[:], idx2[:])

            # ---- x' table: table1 = dinv * x (node rows on partitions) ----
            nch = (NT_X + XCH - 1) // XCH
            for ch in range(nch):
                t0, t1 = ch * XCH, min((ch + 1) * XCH, NT_X)
                w = t1 - t0
                xt = xp.tile([P, XCH, DIN], f32, tag="xt")
                # x rows r = t*128 + p  ->  sbuf [p][t][f]
                nc.sync.dma_start(
                    xt[:, :w, :],
                    x_in.reshape([NT_X, P, DIN])[t0:t1].transpose([1, 0, 2]),
                )
                nc.vector.tensor_mul(
                    xt[:, :w, :], xt[:, :w, :],
                    da_sb[:, t0:t1].unsqueeze(2).to_broadcast([P, w, DIN]),
                )
                nc.sync.dma_start(
                    table1.reshape([NT_X, P, DIN])[t0:t1].transpose([1, 0, 2]),
                    xt[:, :w, :],
                )

            acc = bp.tile([P, T_LOC, DIN], f32)

            def aggregate(idx_sb, table, acc_t):
                for t in range(T_LOC):
                    K = int(Kt[t])
                    G = gp.tile([P, Kmax, DIN], f32, tag="G")
                    for k in range(K):
                        j = int(off[t]) + k
                        nc.gpsimd.indirect_dma_start(
                            out=G[:, k, :], out_offset=None, in_=table[:],
                            in_offset=bass.IndirectOffsetOnAxis(
                                ap=idx_sb[:, j:j + 1], axis=0),
                        )
                    h = K
                    while h > 2:
                        m = h // 2
                        nc.vector.tensor_add(G[:, :m, :], G[:, :m, :], G[:, m:2 * m, :])
                        if h % 2:
                            nc.vector.tensor_add(G[:, 0, :], G[:, 0, :], G[:, 2 * m, :])
                        h = m
                    if h == 2:
                        nc.vector.tensor_add(acc_t[:, t, :], G[:, 0, :], G[:, 1, :])
                    else:
                        nc.vector.tensor_copy(acc_t[:, t, :], G[:, 0, :])

            # ---- layer 1 aggregation ----
            for _r in range(rep):
                aggregate(i1_sb, table1, acc)

            # scale by dinv_d then transpose tiles -> aggT [64, SLOTS]
            aggT = bp.tile([DIN, SLOTS], f32)
            for t in range(T_LOC):
                nc.vector.tensor_mul(
                    acc[:, t, :], acc[:, t, :],
                    dl_sb[:, t:t + 1].to_broadcast([P, DIN]))
                pt = pp.tile([DIN, P], f32, tag="pt")
                nc.tensor.transpose(pt[:], acc[:, t, :], ident[:])
                nc.scalar.activation(aggT[:, t * 128:(t + 1) * 128], pt[:],
                                     mybir.ActivationFunctionType.Copy)

            # ---- q^T = W1^T @ aggT ; relu(q + b1) -> hT [128, SLOTS] ----
            hT = bp.tile([DH, SLOTS], f32)
            MCH = 512
            for m0 in range(0, SLOTS, MCH):
                m1 = min(m0 + MCH, SLOTS)
                pq = pp2.tile([DH, MCH], f32, tag="pq")
                nc.tensor.matmul(pq[:, :m1 - m0], w1_sb[:], aggT[:, m0:m1],
                                 start=True, stop=True)
                nc.scalar.activation(hT[:, m0:m1], pq[:, :m1 - m0],
                                     mybir.ActivationFunctionType.Relu,
                                     bias=b1_sb[:, 0:1])

            # ---- hw^T = W2^T @ hT -> hwT [64, SLOTS] ----
            hwT = bp.tile([DOUT, SLOTS], f32)
            for m0 in range(0, SLOTS, MCH):
                m1 = min(m0 + MCH, SLOTS)
                ph = pp2.tile([DOUT, MCH], f32, tag="ph")
                nc.tensor.matmul(ph[:, :m1 - m0], w2_sb[:], hT[:, m0:m1],
                                 start=True, stop=True)
                nc.scalar.activation(hwT[:, m0:m1], ph[:, :m1 - m0],
                                     mybir.ActivationFunctionType.Copy)

            # ---- transpose back + scale by dinv (source scaling) -> x2_loc ----
            x2 = bp.tile([P, T_LOC, DOUT], f32)
            for t in range(T_LOC):
                px = pp.tile([P, DOUT], f32, tag="px")
                nc.tensor.matmul(px[:], hwT[:, t * 128:(t + 1) * 128],
                                 ident[:DOUT, :DOUT], is_transpose=True)
                nc.scalar.activation(x2[:, t, :], px[:],
                                     mybir.ActivationFunctionType.Copy,
                                     scale=dl_sb[:, t:t + 1])

            # ---- exchange: slice2 -> AllGather -> table2 ----
            nc.sync.dma_start(
                slice2.reshape([T_LOC, P, DOUT]).transpose([1, 0, 2]), x2[:])
            nc.gpsimd.collective_compute(
                "AllGather", mybir.AluOpType.bypass,
                replica_groups=[list(range(NCORES))],
                ins=[slice2.ap().opt()], outs=[table2.ap().opt()],
            )

            # ---- layer 2 aggregation ----
            acc2 = bp.tile([P, T_LOC, DOUT], f32)
            for _r in range(rep):
                aggregate(i2_sb, table2, acc2)

            o_sb = bp.tile([P, T_LOC, DOUT], f32)
            for t in range(T_LOC):
                nc.vector.tensor_mul(
                    o_sb[:, t, :], acc2[:, t, :],
                    dl_sb[:, t:t + 1].to_broadcast([P, DOUT]))
                nc.vector.tensor_add(o_sb[:, t, :], o_sb[:, t, :], b2_sb[:])
            nc.gpsimd.dma_start(out_d.reshape([P, T_LOC, DOUT])[:], o_sb[:])
    nc.compile()
    return nc


def _get(edge_index):
    key = edge_index.tobytes()[:64]  # cheap cache key for repeated calls
    if "prep" not in _cache or _cache.get("key") != key:
        import concourse.bass as bass  # noqa
        prep = _host_prep(edge_index)
        nc = _build_nc(prep["Kt"], prep["off"], prep["SK"])
        _cache.update(prep=prep, nc=nc, key=key)
    return _cache["prep"], _cache["nc"]


def kernel(x, edge_index, W1, b1, W2, b2):
    from concourse.bass_utils import run_bass_kernel_spmd

    prep, nc = _get(np.asarray(edge_index))
    x = np.asarray(x, np.float32)
    xpad = np.zeros((XPAD, DIN), np.float32)
    xpad[:N_NODES] = x
    in_maps = []
    for c in range(NCORES):
        in_maps.append({
            "x_in": xpad,
            "w1": np.asarray(W1, np.float32),
            "b1": np.asarray(b1, np.float32).reshape(DH, 1),
            "w2": np.asarray(W2, np.float32),
            "b2": np.asarray(b2, np.float32).reshape(1, DOUT),
            "idx1": prep["IDX1"][c],
            "idx2": prep["IDX2"][c],
            "dinv_loc": prep["dinv_loc"][c],
            "dinv_all": prep["dinv_all"],
        })
    res = run_bass_kernel_spmd(nc, in_maps, core_ids=list(range(NCORES)))
    out = np.zeros((N_NODES, DOUT), np.float32)
    for c in range(NCORES):
        o = res.results[c]["out"].reshape(P, T_LOC, DOUT)
        perm = prep["perms"][c]
        slots = np.arange(SLOTS)
        valid = perm >= 0
        out[perm[valid]] = o[slots[valid] % 128, slots[valid] // 128]
    return out


# needed inside _build_nc
import concourse.bass as bass  # noqa: E402



# revision 8
# speedup vs baseline: 8.1034x; 8.1034x over previous
"""2-layer GCN (GCNConv x2) on 8 trn2 NeuronCores — transfer-optimized.

out = D^-1/2 (A+I) D^-1/2 (X W) + b per layer; by associativity both
layers aggregate width-64 tables:
  L1: table1 = dinv*x (bf16, AllGathered); agg1 = ELL-gather-sum(table1)
      q^T = W1^T @ (dinv_d*agg1)^T + b1; h = relu(q)
  L2: table2 = dinv*(h @ W2) (bf16, AllGathered); agg2 = gather-sum(table2)
      out = dinv_d*agg2 + b2

Distribution: node blocks of 6272 rows per core (dst-node sharding).
x arrives SHARDED (1/8 per core) and is AllGathered on device, so the
host->device link only carries x once (bf16), not 8 replicas.
Per-edge-ELL index tables depend only on edge_index: they are uploaded
once and kept device-resident across calls. Gathers are BATCHED: one
indirect DMA per (tile, K-chunk) moves up to 64*128 rows.
Output rows are scatter-stored in natural node order (bf16), so the
shard_map concat is already the answer — no host-side permute.

Runner: the jitted shard_map executable is built once and cached; each
kernel() call only transfers x (bf16) + the 4 small weights and fetches
the bf16 output. (Same bass2jax/PJRT execution path that
concourse.bass_utils.run_bass_kernel_spmd uses under axon, minus the
per-call retrace/re-upload.)
"""
import numpy as np
import ml_dtypes

N_NODES = 50000
DIN, DH, DOUT = 64, 128, 64
NCORES = 8
P = 128
T_LOC = 49                    # tiles of 128 dst slots per core
SLOTS = T_LOC * P             # 6272 rows per core
XPAD = NCORES * SLOTS         # 50176 padded node rows
KCAP = 64                     # max gather rows per indirect DMA chunk
MCH = 512                     # slot chunk for the matmul mid-section

BF16 = ml_dtypes.bfloat16

_cache = {}


# ---------------------------------------------------------------- host prep
def _host_prep(edge_index):
    src = edge_index[0].astype(np.int64)
    dst = edge_index[1].astype(np.int64)
    deg = np.bincount(dst, minlength=N_NODES) + 1          # + self loop
    dinv = np.zeros(XPAD, np.float32)
    dinv[:N_NODES] = 1.0 / np.sqrt(deg.astype(np.float64))

    order = np.argsort(dst, kind="stable")
    ssrc = src[order]
    starts = np.searchsorted(dst[order], np.arange(N_NODES))
    ends = np.searchsorted(dst[order], np.arange(N_NODES) + 1)
    cnt = ends - starts                                    # in-degree, no self

    ZROW = XPAD - 1            # known-zero table row (pad node / pad slot)

    perms = np.full((NCORES, SLOTS), -1, np.int64)
    Ks = np.zeros((NCORES, T_LOC), np.int64)
    for c in range(NCORES):
        lo = c * SLOTS
        hi = min(lo + SLOTS, N_NODES)
        nodes = np.arange(lo, hi)
        o = np.argsort(-cnt[nodes], kind="stable")
        perms[c, : hi - lo] = nodes[o]
        ell = np.ones(SLOTS, np.int64)                     # pad slots: K=1
        ell[: hi - lo] = cnt[perms[c, : hi - lo]] + 1      # + self
        Ks[c] = ell.reshape(T_LOC, P).max(axis=1)
    Kt = Ks.max(axis=0)                                    # SPMD-common K
    off = np.concatenate([[0], np.cumsum(Kt)]).astype(np.int64)
    SK = int(off[-1])

    # global slot id of each node (for table2 rows)
    slotpos = np.zeros(XPAD, np.int64)
    for c in range(NCORES):
        p = perms[c]
        v = p >= 0
        slotpos[p[v]] = c * SLOTS + np.where(v)[0]
    slotpos[N_NODES:] = ZROW

    IDX1 = np.full((NCORES, P, SK), ZROW, np.int32)
    IDX2 = np.full((NCORES, P, SK), ZROW, np.int32)
    dinv_loc = np.zeros((NCORES, P, T_LOC), np.float32)
    oidx = np.full((NCORES, P, T_LOC), SLOTS, np.int32)    # pad -> OOB-skip
    scol = np.repeat(off[:-1], P).reshape(T_LOC, P).T      # [P,T] col base
    for c in range(NCORES):
        perm = perms[c]
        v = perm >= 0
        sl = np.where(v)[0]                                # valid slots
        nd = perm[sl]
        pp, tt = sl % P, sl // P
        dinv_loc[c, pp, tt] = dinv[nd]
        oidx[c, pp, tt] = (nd - c * SLOTS).astype(np.int32)
        lens = cnt[nd]
        tot = int(lens.sum())
        rep_sl = np.repeat(sl, lens)
        kwi = np.arange(tot) - np.repeat(np.cumsum(lens) - lens, lens)
        epos = np.repeat(starts[nd], lens) + kwi
        rows = rep_sl % P
        cols = off[rep_sl // P] + kwi
        sv = ssrc[epos]
        IDX1[c, rows, cols] = sv
        IDX2[c, rows, cols] = slotpos[sv]
        # self loop right after the neighbours
        IDX1[c, pp, off[tt] + lens] = nd
        IDX2[c, pp, off[tt] + lens] = slotpos[nd]

    dinv_sh = np.zeros((NCORES, P, T_LOC), np.float32)     # natural order
    for c in range(NCORES):
        dinv_sh[c] = dinv[c * SLOTS:(c + 1) * SLOTS].reshape(T_LOC, P).T
    return dict(Kt=Kt.astype(int), off=off.astype(int), SK=SK, IDX1=IDX1,
                IDX2=IDX2, dinv_loc=dinv_loc, dinv_sh=dinv_sh, oidx=oidx)


# ---------------------------------------------------------------- bass kernel
def _build_nc(Kt, off, SK):
    import concourse.bass as bass
    import concourse.bacc as bacc
    import concourse.mybir as mybir
    import concourse.tile as tile
    from concourse.masks import make_identity

    f32 = mybir.dt.float32
    bf16 = mybir.dt.bfloat16
    i32 = mybir.dt.int32
    nc = bacc.Bacc("TRN2", target_bir_lowering=False)
    x_in = nc.dram_tensor("x_in", [SLOTS, DIN], bf16, kind="ExternalInput")
    w1 = nc.dram_tensor("w1", [DIN, DH], f32, kind="ExternalInput")
    b1 = nc.dram_tensor("b1", [DH, 1], f32, kind="ExternalInput")
    w2 = nc.dram_tensor("w2", [DH, DOUT], f32, kind="ExternalInput")
    b2 = nc.dram_tensor("b2", [1, DOUT], f32, kind="ExternalInput")
    idx1 = nc.dram_tensor("idx1", [P, SK], i32, kind="ExternalInput")
    idx2 = nc.dram_tensor("idx2", [P, SK], i32, kind="ExternalInput")
    dinv_loc_d = nc.dram_tensor("dinv_loc", [P, T_LOC], f32, kind="ExternalInput")
    dinv_sh_d = nc.dram_tensor("dinv_sh", [P, T_LOC], f32, kind="ExternalInput")
    oidx_d = nc.dram_tensor("oidx", [P, T_LOC], i32, kind="ExternalInput")
    out_d = nc.dram_tensor("out", [SLOTS, DIN], bf16, kind="ExternalOutput")

    slice1 = nc.dram_tensor("slice1", [SLOTS, DIN], bf16)
    table1 = nc.dram_tensor("table1", [XPAD, DIN], bf16, addr_space="Shared")
    slice2 = nc.dram_tensor("slice2", [SLOTS, DOUT], bf16)
    table2 = nc.dram_tensor("table2", [XPAD, DOUT], bf16, addr_space="Shared")

    with tile.TileContext(nc) as tc:
        with (
            tc.tile_pool(name="const", bufs=1) as cp,
            tc.tile_pool(name="g", bufs=3) as gp,
            tc.tile_pool(name="g2", bufs=2) as g2p,
            tc.tile_pool(name="big", bufs=1) as bp,
            tc.tile_pool(name="mid", bufs=2) as mp,
            tc.tile_pool(name="pt", bufs=2, space="PSUM") as ptp,
            tc.tile_pool(name="pq", bufs=2, space="PSUM") as pqp,
            tc.tile_pool(name="ph", bufs=2, space="PSUM") as php,
            tc.tile_pool(name="px", bufs=2, space="PSUM") as pxp,
        ):
            ident = cp.tile([P, P], f32)
            make_identity(nc, ident[:])
            w1_sb = cp.tile([DIN, DH], f32)
            w2_sb = cp.tile([DH, DOUT], f32)
            b1_sb = cp.tile([DH, 1], f32)
            b2_sb = cp.tile([P, DOUT], f32)
            dl_sb = cp.tile([P, T_LOC], f32)
            ds_sb = cp.tile([P, T_LOC], bf16)
            oi_sb = cp.tile([P, T_LOC], i32)
            i1_sb = cp.tile([P, SK], i32)
            i2_sb = cp.tile([P, SK], i32)
            nc.sync.dma_start(w1_sb[:], w1[:])
            nc.sync.dma_start(w2_sb[:], w2[:])
            nc.sync.dma_start(b1_sb[:], b1[:])
            nc.sync.dma_start(b2_sb[:], b2[:].to_broadcast([P, DOUT]))
            nc.sync.dma_start(dl_sb[:], dinv_loc_d[:])
            nc.gpsimd.dma_start(ds_sb[:], dinv_sh_d[:])  # f32 -> bf16 cast
            nc.sync.dma_start(oi_sb[:], oidx_d[:])
            nc.sync.dma_start(i1_sb[:], idx1[:])
            nc.sync.dma_start(i2_sb[:], idx2[:])

            # ---- prep: slice1 = bf16(dinv * x_shard), AllGather -> table1
            xt = bp.tile([P, T_LOC, DIN], bf16)
            nc.sync.dma_start(
                xt[:], x_in.reshape([T_LOC, P, DIN]).transpose([1, 0, 2]))
            xs = bp.tile([P, T_LOC, DIN], bf16)
            nc.vector.tensor_mul(
                xs[:], xt[:],
                ds_sb[:].unsqueeze(2).to_broadcast([P, T_LOC, DIN]))
            nc.sync.dma_start(
                slice1.reshape([T_LOC, P, DIN]).transpose([1, 0, 2]), xs[:])
            nc.gpsimd.collective_compute(
                "AllGather", mybir.AluOpType.bypass,
                replica_groups=[list(range(NCORES))],
                ins=[slice1.ap().opt()], outs=[table1.ap().opt()],
            )

            # ---- ELL gather (one row per partition per DMA) + free-dim
            # reduce with fp32 accumulate.  NOTE: multi-column offset APs in
            # one indirect DMA mis-gather on real HW; per-column is the
            # proven form.
            KMAXT = int(max(Kt))

            def aggregate(idx_sb, table, acc_t, width):
                for t in range(T_LOC):
                    K = int(Kt[t])
                    G = gp.tile([P, KMAXT, width], bf16, tag="G")
                    for k in range(K):
                        j = int(off[t]) + k
                        nc.gpsimd.indirect_dma_start(
                            out=G[:, k, :], out_offset=None, in_=table[:],
                            in_offset=bass.IndirectOffsetOnAxis(
                                ap=idx_sb[:, j:j + 1], axis=0),
                        )
                    nc.vector.tensor_reduce(
                        acc_t[:, t, :], G[:, :K, :].transpose([0, 2, 1]),
                        axis=mybir.AxisListType.X,
                        op=mybir.AluOpType.add)

            # ---- layer 1 ----
            acc = bp.tile([P, T_LOC, DIN], f32)
            aggregate(i1_sb, table1, acc, DIN)

            # mid-section in slot chunks: transpose -> W1 -> relu -> W2 ->
            # transpose back, scaled by dinv_d on both ends.
            x2 = bp.tile([P, T_LOC, DOUT], bf16)
            for m0 in range(0, SLOTS, MCH):
                m1 = min(m0 + MCH, SLOTS)
                w = m1 - m0
                nt = (w + P - 1) // P
                aggT = mp.tile([DIN, MCH], f32, tag="aggT")
                for i in range(nt):
                    t = m0 // P + i
                    nc.vector.tensor_mul(
                        acc[:, t, :], acc[:, t, :],
                        dl_sb[:, t:t + 1].to_broadcast([P, DIN]))
                    pt = ptp.tile([DIN, P], f32, tag="pt")
                    nc.tensor.transpose(pt[:], acc[:, t, :], ident[:])
                    nc.scalar.activation(aggT[:, i * P:(i + 1) * P], pt[:],
                                         mybir.ActivationFunctionType.Copy)
                pq = pqp.tile([DH, MCH], f32, tag="pq")
                nc.tensor.matmul(pq[:, :w], w1_sb[:], aggT[:, :w],
                                 start=True, stop=True)
                hT = mp.tile([DH, MCH], f32, tag="hT")
                nc.scalar.activation(hT[:, :w], pq[:, :w],
                                     mybir.ActivationFunctionType.Relu,
                                     bias=b1_sb[:, 0:1])
                ph = php.tile([DOUT, MCH], f32, tag="ph")
                nc.tensor.matmul(ph[:, :w], w2_sb[:], hT[:, :w],
                                 start=True, stop=True)
                hw = mp.tile([DOUT, MCH], f32, tag="hw")
                nc.scalar.activation(hw[:, :w], ph[:, :w],
                                     mybir.ActivationFunctionType.Copy)
                for i in range(nt):
                    t = m0 // P + i
                    px = pxp.tile([P, DOUT], f32, tag="px")
                    nc.tensor.matmul(px[:], hw[:, i * P:(i + 1) * P],
                                     ident[:DOUT, :DOUT], is_transpose=True)
                    nc.scalar.activation(x2[:, t, :], px[:],
                                         mybir.ActivationFunctionType.Copy,
                                         scale=dl_sb[:, t:t + 1])

            # ---- exchange + layer 2 ----
            nc.sync.dma_start(
                slice2.reshape([T_LOC, P, DOUT]).transpose([1, 0, 2]), x2[:])
            nc.gpsimd.collective_compute(
                "AllGather", mybir.AluOpType.bypass,
                replica_groups=[list(range(NCORES))],
                ins=[slice2.ap().opt()], outs=[table2.ap().opt()],
            )

            acc2 = bp.tile([P, T_LOC, DOUT], f32)
            aggregate(i2_sb, table2, acc2, DOUT)

            o_sb = bp.tile([P, T_LOC, DOUT], bf16)
            for t in range(T_LOC):
                nc.vector.tensor_mul(
                    acc2[:, t, :], acc2[:, t, :],
                    dl_sb[:, t:t + 1].to_broadcast([P, DOUT]))
                nc.vector.tensor_add(o_sb[:, t, :], acc2[:, t, :], b2_sb[:])
                # scatter rows to natural node order (pad slots OOB-skipped)
                nc.gpsimd.indirect_dma_start(
                    out=out_d[:],
                    out_offset=bass.IndirectOffsetOnAxis(
                        ap=oi_sb[:, t:t + 1], axis=0),
                    in_=o_sb[:, t, :], in_offset=None,
                    bounds_check=SLOTS - 1, oob_is_err=False,
                )
    nc.compile()
    return nc


# ---------------------------------------------------------------- runner
class _Runner:
    """jit-once executor for the SPMD bass kernel (axon/PJRT path)."""

    def __init__(self, nc):
        import jax
        import jax.numpy as jnp
        from jax.sharding import Mesh, PartitionSpec, NamedSharding
        from jax.experimental.shard_map import shard_map
        import concourse.mybir as mybir
        from concourse import bass2jax
        from concourse.bass2jax import _bass_exec_p, partition_id_tensor

        bass2jax.install_neuronx_cc_hook()
        self.jax = jax
        self.np = np
        in_names, out_names, out_avals, zero_shapes = [], [], [], []
        partition_name = (
            nc.partition_id_tensor.name if nc.partition_id_tensor else None)
        for alloc in nc.m.functions[0].allocations:
            if not isinstance(alloc, mybir.MemoryLocationSet):
                continue
            name = alloc.memorylocations[0].name
            if alloc.kind == "ExternalInput":
                if name != partition_name:
                    in_names.append(name)
            elif alloc.kind == "ExternalOutput":
                shape = tuple(alloc.tensor_shape)
                dtype = mybir.dt.np(alloc.dtype)
                out_names.append(name)
                out_avals.append(jax.core.ShapedArray(shape, dtype))
                zero_shapes.append((shape, dtype))
        self.in_names = list(in_names)
        self.out_names = out_names
        n_params = len(in_names)
        n_outs = len(out_avals)
        all_names = in_names + out_names
        if partition_name is not None:
            all_names.append(partition_name)

        def _body(*args):
            operands = list(args)
            if partition_name is not None:
                operands.append(partition_id_tensor())
            outs = _bass_exec_p.bind(
                *operands,
                out_avals=tuple(out_avals),
                in_names=tuple(all_names),
                out_names=tuple(out_names),
                lowering_input_output_aliases=(),
                sim_require_finite=True,
                sim_require_nnan=True,
                nc=nc,
            )
            return tuple(outs)

        devices = jax.devices()[:NCORES]
        self.mesh = Mesh(np.asarray(devices), ("core",))
        self.sharding = NamedSharding(self.mesh, PartitionSpec("core"))
        in_specs = (PartitionSpec("core"),) * (n_params + n_outs)
        out_specs = (PartitionSpec("core"),) * n_outs
        donate = tuple(range(n_params, n_params + n_outs))
        self.jitted = jax.jit(
            shard_map(_body, mesh=self.mesh, in_specs=in_specs,
                      out_specs=out_specs, check_rep=False),
            donate_argnums=donate, keep_unused=True)

        def _zeros():
            return tuple(
                jnp.zeros((NCORES * s[0], *s[1:]), d) for s, d in zero_shapes)

        self.zeros_fn = jax.jit(
            _zeros, out_shardings=(self.sharding,) * n_outs)
        self.static = {}            # name -> device-resident jax.Array

    def put_static(self, name, global_np):
        self.static[name] = self.jax.device_put(global_np, self.sharding)

    def __call__(self, dyn):
        args = [dyn[n] if n in dyn else self.static[n] for n in self.in_names]
        outs = self.jitted(*args, *self.zeros_fn())
        return {n: outs[i] for i, n in enumerate(self.out_names)}


def _edge_key(edge_index):
    a = np.ascontiguousarray(edge_index)
    return (a.shape, a.dtype.str, a.tobytes()[:4096],
            a.reshape(-1)[:: max(1, a.size // 8192)].tobytes())


def _get(edge_index):
    key = _edge_key(edge_index)
    if _cache.get("key") != key:
        prep = _host_prep(np.asarray(edge_index))
        nc = _build_nc(prep["Kt"], prep["off"], prep["SK"])
        runner = _Runner(nc)
        for name, arr in (
            ("idx1", prep["IDX1"]), ("idx2", prep["IDX2"]),
            ("dinv_loc", prep["dinv_loc"]), ("dinv_sh", prep["dinv_sh"]),
            ("oidx", prep["oidx"]),
        ):
            runner.put_static(name, np.concatenate(list(arr), axis=0))
        _cache.clear()
        _cache.update(prep=prep, nc=nc, runner=runner, key=key)
    return _cache["prep"], _cache["runner"]


def kernel(x, edge_index, W1, b1, W2, b2):
    prep, runner = _get(edge_index)
    xpad = np.zeros((XPAD, DIN), BF16)
    xpad[:N_NODES] = np.asarray(x, np.float32)
    w1 = np.asarray(W1, np.float32)
    b1v = np.asarray(b1, np.float32).reshape(DH, 1)
    w2 = np.asarray(W2, np.float32)
    b2v = np.asarray(b2, np.float32).reshape(1, DOUT)
    dyn = {
        "x_in": xpad,
        "w1": np.concatenate([w1] * NCORES, axis=0),
        "b1": np.concatenate([b1v] * NCORES, axis=0),
        "w2": np.concatenate([w2] * NCORES, axis=0),
        "b2": np.concatenate([b2v] * NCORES, axis=0),
    }
    outs = runner(dyn)
    full = np.asarray(outs["out"])          # [XPAD, DOUT] bf16
    return full[:N_NODES].astype(np.float32)
